# revision 25
# baseline (speedup 1.0000x reference)
"""Trainium2 Bass kernel for EvolveGCN-O forward (GCN message passing).

Math (reference):
    h   = x @ Wp + bp
    W   = LSTM-evolved weight from initial_weight (one step, h0=c0=IW)
    hw  = h @ W
    out = D^-1/2 (A+I) D^-1/2 hw + b_gcn

Factored for the kernel:
    out[d] = dinv[d] * (sum_{e: dst=d} dinv[src_e] * x[src_e]) @ (Wp @ W)
             + s2[d]*dinv[d]*(bp @ W) + b_gcn
with s2[d] = sum_{e in(d)} dinv[src_e] (self loops included as edges).

Distribution: dst nodes sharded contiguously over 8 NeuronCores (1280 each).
The aggregation sum_{e:dst=d} dinv[src]*x[src] is computed as dense matmuls
against a per-core count matrix S[src, dst_local] (fp8, exact small ints;
self loops folded in):  xagg[feat, dst] = sum_r xs_r^T @ S_r  over the 80
src ranks of 128.  Each rank's row of the stream tensor packs
[S (1280 dst) | x_hi (128) | x_lo (128)] in fp8, so one HBM stream on the
sync+scalar HWDGE queues feeds everything in arrival order.  The
TensorEngine consumes rank pairs with fp8 DoubleRow matmuls (k=256 per
pass, three 512/512/256-col PSUM chains); x is split hi+lo fp8 so near-
fp16 accuracy is kept (the lo correction is skipped for the last 4 rank
pairs, trading a little accuracy for time).  No SWDGE gather, no gpsimd
library swap.  The tiny evolved-weight LSTM is replicated on every core.
"""

import numpy as np

N_NODES = 10000
N_EDGES = 320000
IN_DIM = 128
HID = 256
M = 8                    # NeuronCores
NP = 10240               # padded node count (mult of 128)
RANKS = NP // 128        # 80 src ranks
NPAIR = RANKS // 2       # 40 DoubleRow rank pairs
NPC = NP // M            # 1280 dsts per core
NGRP = NPC // 128        # 10 dst groups of 128 per core
RW = NPC + 2 * 128       # stream cols per rank: S | x_hi | x_lo (chunk-blocked)
CHUNKS = [2, 2] + [4] * 19           # ranks per stream DMA (sum = 80)
CSTART = [sum(CHUNKS[:i]) for i in range(len(CHUNKS))]
GATES_AFTER = 8          # rank PAIRS before LSTM gate matmuls
DROP_LO = frozenset(range(36, 40))   # pairs whose x_lo correction is skipped

_cache = {}


def _build_module():
    import bisect
    import concourse.bacc as bacc
    import concourse.mybir as mybir
    import concourse.tile as tile

    nc = bacc.Bacc("TRN2", target_bir_lowering=False, debug=False,
                   num_devices=M)
    f32, f16, f8 = mybir.dt.float32, mybir.dt.float16, mybir.dt.float8e4
    DR = mybir.MatmulPerfMode.DoubleRow
    DRSI = mybir.MatmulPerfMode.DoubleRowSwInterleave

    # ---- DRAM inputs ----
    st_in = nc.dram_tensor("stream", [128, RANKS * RW], f8, kind="ExternalInput").ap()
    wsum_in = nc.dram_tensor("wsumT", [256, 1024], f16, kind="ExternalInput").ap()
    bsum_in = nc.dram_tensor("bsum", [1, 1024], f16, kind="ExternalInput").ap()
    iw_in = nc.dram_tensor("IW", [256, 256], f32, kind="ExternalInput").ap()
    iwt_in = nc.dram_tensor("IWT", [256, 256], f16, kind="ExternalInput").ap()
    wpt_in = nc.dram_tensor("WpT", [256, 128], f16, kind="ExternalInput").ap()
    bp_in = nc.dram_tensor("bp_col", [256, 1], f16, kind="ExternalInput").ap()
    bgcn_in = nc.dram_tensor("b_gcn", [1, 256], f16, kind="ExternalInput").ap()
    ones_in = nc.dram_tensor("ones_row", [1, 128], f16, kind="ExternalInput").ap()
    sd_in = nc.dram_tensor("sd_rows", [2, NPC], f16, kind="ExternalInput").ap()
    dcol_in = nc.dram_tensor("dinv_col", [128, NGRP], f32, kind="ExternalInput").ap()

    out_t = nc.dram_tensor("out", [NPC, HID], f16, kind="ExternalOutput").ap()

    Sig = mybir.ActivationFunctionType.Sigmoid
    Tanh = mybir.ActivationFunctionType.Tanh

    with tile.TileContext(nc) as tc:
        with (
            tc.tile_pool(name="persist", bufs=1) as pp,
            tc.tile_pool(name="schunks", bufs=len(CHUNKS)) as spool,
            tc.tile_pool(name="stage", bufs=1) as stpool,
            tc.tile_pool(name="fin", bufs=4) as fpool,
            tc.tile_pool(name="psacc_a", bufs=1, space="PSUM") as psacc_a,
            tc.tile_pool(name="psacc_b", bufs=1, space="PSUM") as psacc_b,
            tc.tile_pool(name="psacc_c", bufs=1, space="PSUM") as psacc_c,
            tc.tile_pool(name="psg", bufs=2, space="PSUM") as psg,
            tc.tile_pool(name="psl", bufs=1, space="PSUM") as psl,
        ):
            # ---------- input DMAs ----------
            # sync / scalar: stream chunks alternating (sync also takes the
            # LSTM gate weights early and the out writes late)
            # gpsimd: remaining small tensors
            wsum = pp.tile([128, 2, 1024], f16)
            iwt = pp.tile([128, 2, 256], f16)
            iw = pp.tile([128, 2, 256], f32)
            wpt = pp.tile([128, 2, 128], f16)
            bp_c = pp.tile([128, 2, 1], f16)
            bsum = pp.tile([1, 1024], f16)
            ones = pp.tile([1, 128], f16)
            # bb rows: bp@W (computed later), b_gcn (DMA'd)
            bb = pp.tile([2, 256], f16)
            sd = pp.tile([2, NPC], f16)          # rows: s2, 1/dinv
            dcol = pp.tile([128, NGRP], f32)

            schunks = [None] * len(CHUNKS)

            def emit_s_chunk(k, eng):
                sz = CHUNKS[k]
                r0 = CSTART[k]
                sch = spool.tile([128, sz * RW], f8, tag="schunk",
                                 name=f"schunk{k}")
                eng.dma_start(out=sch[:], in_=st_in[:, r0 * RW:(r0 + sz) * RW])
                schunks[k] = sch

            emit_s_chunk(0, nc.sync)
            emit_s_chunk(1, nc.scalar)
            emit_s_chunk(2, nc.sync)
            emit_s_chunk(3, nc.scalar)
            for t_, src_ in ((iwt, iwt_in), (wsum, wsum_in)):
                nc.sync.dma_start(
                    out=t_[:], in_=src_.rearrange("(k p) c -> p k c", p=128))
            nc.sync.dma_start(out=bsum[:], in_=bsum_in[:])
            for i, k in enumerate(range(4, len(CHUNKS))):
                emit_s_chunk(k, nc.scalar if i % 2 == 0 else nc.sync)

            for t_, src_ in ((iw, iw_in), (wpt, wpt_in), (bp_c, bp_in)):
                nc.gpsimd.dma_start(
                    out=t_[:], in_=src_.rearrange("(k p) c -> p k c", p=128))
            for t_, src_ in ((ones, ones_in), (sd, sd_in), (dcol, dcol_in)):
                nc.gpsimd.dma_start(out=t_[:], in_=src_[:])
            nc.gpsimd.dma_start(out=bb[1:2, :], in_=bgcn_in[:])

            # ---------- LSTM weight evolution (emitted mid rank loop) -------
            w_ev = pp.tile([128, 2, 256], f16)   # evolved GCN weight W

            def emit_gates(ic):
                gpsum = psl.tile([128, 1024], f32, space="PSUM", tag="gates",
                                 name=f"gates{ic}")
                for h in range(2):
                    gs = slice(512 * h, 512 * (h + 1))
                    nc.tensor.matmul(out=gpsum[:, gs], lhsT=ones[:, :],
                                     rhs=bsum[:, gs], start=True, stop=False)
                    nc.tensor.matmul(out=gpsum[:, gs],
                                     lhsT=iwt[:, 0, 128 * ic:128 * (ic + 1)],
                                     rhs=wsum[:, 0, gs], start=False, stop=False)
                    nc.tensor.matmul(out=gpsum[:, gs],
                                     lhsT=iwt[:, 1, 128 * ic:128 * (ic + 1)],
                                     rhs=wsum[:, 1, gs], start=False, stop=True)
                return gpsum

            def emit_lstm_post(gpsum, ic):
                si = stpool.tile([128, 256], f32, tag="si", name=f"si{ic}")
                sf = stpool.tile([128, 256], f32, tag="sf", name=f"sf{ic}")
                so = stpool.tile([128, 256], f32, tag="so", name=f"so{ic}")
                tg = stpool.tile([128, 256], f32, tag="tg", name=f"tg{ic}")
                nc.scalar.activation(out=si[:], in_=gpsum[:, 0:256], func=Sig)
                nc.scalar.activation(out=sf[:], in_=gpsum[:, 256:512], func=Sig)
                nc.scalar.activation(out=so[:], in_=gpsum[:, 768:1024], func=Sig)
                nc.scalar.activation(out=tg[:], in_=gpsum[:, 512:768], func=Tanh)
                c1 = stpool.tile([128, 256], f32, tag="c1", name=f"c1_{ic}")
                nc.vector.tensor_tensor(out=c1[:], in0=sf[:], in1=iw[:, ic, :],
                                        op=mybir.AluOpType.mult)
                c2 = stpool.tile([128, 256], f32, tag="c2", name=f"c2_{ic}")
                nc.vector.tensor_tensor(out=c2[:], in0=si[:], in1=tg[:],
                                        op=mybir.AluOpType.mult)
                cc = stpool.tile([128, 256], f32, tag="cc", name=f"cc{ic}")
                nc.vector.tensor_tensor(out=cc[:], in0=c1[:], in1=c2[:],
                                        op=mybir.AluOpType.add)
                tcc = stpool.tile([128, 256], f32, tag="tcc", name=f"tcc{ic}")
                nc.scalar.activation(out=tcc[:], in_=cc[:], func=Tanh)
                nc.vector.tensor_tensor(out=w_ev[:, ic, :], in0=so[:],
                                        in1=tcc[:], op=mybir.AluOpType.mult)

            def emit_wpw():
                wpw = pp.tile([128, 256], f16)       # Wp @ W
                wp_ps = psg.tile([128, HID], f32, space="PSUM", tag="ops",
                                 name="wp_ps")
                nc.tensor.matmul(out=wp_ps[:], lhsT=wpt[:, 0, :], rhs=w_ev[:, 0, :],
                                 start=True, stop=False)
                nc.tensor.matmul(out=wp_ps[:], lhsT=wpt[:, 1, :], rhs=w_ev[:, 1, :],
                                 start=False, stop=True)
                nc.vector.tensor_copy(out=wpw[:], in_=wp_ps[:])
                bp_ps = psg.tile([128, HID], f32, space="PSUM", tag="ops",
                                 name="bp_ps")
                nc.tensor.matmul(out=bp_ps[0:1, :], lhsT=bp_c[:, 0, :],
                                 rhs=w_ev[:, 0, :], start=True, stop=False)
                nc.tensor.matmul(out=bp_ps[0:1, :], lhsT=bp_c[:, 1, :],
                                 rhs=w_ev[:, 1, :], start=False, stop=True)
                nc.vector.tensor_copy(out=bb[0:1, :], in_=bp_ps[0:1, :])
                return wpw

            def emit_final(g, accs, wpw):
                xagg = fpool.tile([128, 128], f16, tag="xagg", name=f"xagg{g}")
                ti = 0 if g < 4 else (1 if g < 8 else 2)
                acc, gg = accs[ti], g - (0, 4, 8)[ti]
                nc.vector.tensor_copy(
                    out=xagg[:], in_=acc[:, 128 * gg:128 * (gg + 1)])
                ops = psg.tile([128, HID], f32, space="PSUM", tag="ops",
                               name=f"ops{g}")
                ds = slice(128 * g, 128 * (g + 1))
                nc.tensor.matmul(out=ops[:], lhsT=sd[:, ds], rhs=bb[:],
                                 start=True, stop=False)
                nc.tensor.matmul(out=ops[:], lhsT=xagg[:], rhs=wpw[:],
                                 start=False, stop=True)
                orow = fpool.tile([128, HID], f16, tag="orow", name=f"orow{g}")
                nc.scalar.activation(out=orow[:], in_=ops[:],
                                     func=mybir.ActivationFunctionType.Copy,
                                     scale=dcol[:, g:g + 1])
                nc.sync.dma_start(
                    out=out_t.rearrange("(g p) h -> g p h", p=128)[g],
                    in_=orow[:],
                )

            # ---------- main: DoubleRow pass over 40 src rank pairs ---------
            acc_a = psacc_a.tile([128, 512], f32, space="PSUM", tag="acc_a")
            acc_b = psacc_b.tile([128, 512], f32, space="PSUM", tag="acc_b")
            acc_c = psacc_c.tile([128, 256], f32, space="PSUM", tag="acc_c")
            TILES = [(0, 512, acc_a), (512, 512, acc_b), (1024, 256, acc_c)]
            gp0 = gp1 = None
            wpw = None
            for pr in range(NPAIR):
                if pr == GATES_AFTER:
                    gp0 = emit_gates(0)
                if pr == GATES_AFTER + 8:
                    emit_lstm_post(gp0, 0)
                    gp1 = emit_gates(1)
                if pr == GATES_AFTER + 16:
                    emit_lstm_post(gp1, 1)
                if pr == GATES_AFTER + 20:
                    wpw = emit_wpw()
                r = 2 * pr
                k = bisect.bisect_right(CSTART, r) - 1
                jj = r - CSTART[k]
                sz = CHUNKS[k]
                sch = schunks[k]
                srow = sch[:, 0:sz * NPC].rearrange("p (j c) -> p j c", c=NPC)
                last = pr == NPAIR - 1
                last_hl = 0 if NPAIR - 1 in DROP_LO else 1
                for hl in range(2):
                    if hl == 1 and pr in DROP_LO:
                        continue
                    xb = sz * NPC + hl * sz * 128
                    x_t = sch[:, xb + jj * 128:xb + (jj + 2) * 128] \
                        .rearrange("p (j c) -> p j c", c=128)
                    for ti, (c0, w, acc) in enumerate(TILES):
                        mm = nc.tensor.matmul(
                            out=acc[:],
                            lhsT=x_t,
                            rhs=srow[:, jj:jj + 2, c0:c0 + w],
                            start=(pr == 0 and hl == 0),
                            stop=(last and hl == last_hl),
                            perf_mode=DR,
                        )
                        if ti > 0:
                            # same stationary x pair as the previous matmul —
                            # skip the redundant PE weight reload
                            mm.ldweights = False

            for g in range(NGRP):
                emit_final(g, (acc_a, acc_b, acc_c), wpw)

    nc.compile()
    return nc


def _preprocess(edge_index):
    """Host-side: degree norms, per-core fp8 count matrices (self loops in)."""
    src = np.asarray(edge_index[0], dtype=np.int64)
    dst = np.asarray(edge_index[1], dtype=np.int64)
    deg = np.bincount(dst, minlength=N_NODES).astype(np.float64) + 1.0
    dinv = (1.0 / np.sqrt(deg)).astype(np.float32)

    # s2[d] = sum over in-edges of dinv[src], self loop included
    s2 = (np.bincount(dst, weights=dinv[src].astype(np.float64),
                      minlength=N_NODES) + dinv.astype(np.float64)).astype(np.float32)
    return dinv, s2, src, dst


LAST_RESULT = None


def kernel(x, edge_index, Wp, bp, W_ih, W_hh, b_ih, b_hh, initial_weight, b_gcn):
    global LAST_RESULT
    import ml_dtypes
    from concourse.bass_utils import run_bass_kernel_spmd

    f8 = ml_dtypes.float8_e4m3
    x = np.asarray(x, np.float32)
    Wp = np.asarray(Wp, np.float32)
    bp = np.asarray(bp, np.float32)
    W_ih = np.asarray(W_ih, np.float32)
    W_hh = np.asarray(W_hh, np.float32)
    b_ih = np.asarray(b_ih, np.float32)
    b_hh = np.asarray(b_hh, np.float32)
    initial_weight = np.asarray(initial_weight, np.float32)
    b_gcn = np.asarray(b_gcn, np.float32)
    assert x.shape == (N_NODES, IN_DIM)

    dinv, s2, src, dst = _preprocess(edge_index)

    if "nc" not in _cache:
        _cache["nc"] = _build_module()
    nc = _cache["nc"]

    # host pre-scales x rows by dinv[src]; hi+lo fp8 token tables
    xp = np.zeros((NP, IN_DIM), np.float32)
    xp[:N_NODES] = x * dinv[:, None]
    xp_t = np.ascontiguousarray(
        xp.reshape(RANKS, 128, IN_DIM).transpose(1, 0, 2))  # [128, R, 128]
    xhi = xp_t.astype(f8)
    xlo = (xp_t - xhi.astype(np.float32)).astype(f8)

    def swi(xq):
        # DoubleRowSwInterleave weight layout per rank pair (A, B):
        # flat cols [A_127, B_127, A_126, B_126, ..., A_0, B_0]
        pq = xq.reshape(128, NPAIR, 2, 128)
        o = np.empty((128, NPAIR, 256), xq.dtype)
        o[:, :, 0::2] = pq[:, :, 0, ::-1]
        o[:, :, 1::2] = pq[:, :, 1, ::-1]
        return o

    xhi_swi = swi(xhi)
    xlo_swi = swi(xlo)

    core = dst // NPC
    dloc = dst - core * NPC

    wsumT = np.ascontiguousarray((W_ih + W_hh).T).astype(np.float16)
    bsum = (b_ih + b_hh).reshape(1, -1).astype(np.float16)

    s2p = np.zeros(NP, np.float32)
    s2p[:N_NODES] = s2
    drip = np.zeros(NP, np.float32)
    drip[:N_NODES] = 1.0 / dinv
    dlocp = np.zeros(NP, np.float32)
    dlocp[:N_NODES] = dinv

    shared = {
        "wsumT": wsumT,
        "bsum": bsum,
        "IW": initial_weight,
        "IWT": np.ascontiguousarray(initial_weight.T).astype(np.float16),
        "WpT": np.ascontiguousarray(Wp.T).astype(np.float16),
        "bp_col": np.ascontiguousarray(bp.reshape(-1, 1)).astype(np.float16),
        "b_gcn": b_gcn.reshape(1, -1).astype(np.float16),
        "ones_row": np.ones((1, 128), np.float16),
    }
    in_maps = []
    for c in range(M):
        m = core == c
        flat = src[m] * NPC + dloc[m]
        cnt = np.bincount(flat, minlength=NP * NPC)
        d0, d1 = c * NPC, min((c + 1) * NPC, N_NODES)
        dd = np.arange(d0, d1, dtype=np.int64)
        cnt[dd * NPC + (dd - d0)] += 1     # self loops
        sc = cnt.reshape(RANKS, 128, NPC).transpose(1, 0, 2)  # [128, R, NPC]
        sc8 = sc.astype(f8)
        stream = np.empty((128, RANKS * RW), f8)
        for k, sz in enumerate(CHUNKS):
            r0 = CSTART[k]
            base = r0 * RW
            ns = sz * NPC
            stream[:, base:base + ns] = sc8[:, r0:r0 + sz, :].reshape(128, ns)
            stream[:, base + ns:base + ns + sz * 128] = \
                xhi[:, r0:r0 + sz, :].reshape(128, sz * 128)
            stream[:, base + ns + sz * 128:base + sz * RW] = \
                xlo[:, r0:r0 + sz, :].reshape(128, sz * 128)
        sl = slice(c * NPC, (c + 1) * NPC)
        in_maps.append({
            **shared,
            "stream": stream,
            "sd_rows": np.stack([s2p[sl], drip[sl]]).astype(np.float16),
            "dinv_col": np.ascontiguousarray(
                dlocp[sl].reshape(NGRP, 128).T),
        })

    res = run_bass_kernel_spmd(nc, in_maps, list(range(M)))
    LAST_RESULT = res

    out = np.empty((N_NODES, HID), np.float32)
    for c in range(M):
        d0, d1 = c * NPC, min((c + 1) * NPC, N_NODES)
        out[d0:d1] = res.results[c]["out"][:d1 - d0].astype(np.float32)
    return out


# revision 26
# speedup vs baseline: 1.0016x; 1.0016x over previous
"""Trainium2 Bass kernel for EvolveGCN-O forward (GCN message passing).

Math (reference):
    h   = x @ Wp + bp
    W   = LSTM-evolved weight from initial_weight (one step, h0=c0=IW)
    hw  = h @ W
    out = D^-1/2 (A+I) D^-1/2 hw + b_gcn

Factored for the kernel:
    out[d] = dinv[d] * (sum_{e: dst=d} dinv[src_e] * x[src_e]) @ (Wp @ W)
             + s2[d]*dinv[d]*(bp @ W) + b_gcn
with s2[d] = sum_{e in(d)} dinv[src_e] (self loops included as edges).

Distribution: dst nodes sharded contiguously over 8 NeuronCores (1280 each).
The aggregation sum_{e:dst=d} dinv[src]*x[src] is computed as dense matmuls
against a per-core count matrix S[src, dst_local] (fp8, exact small ints;
self loops folded in):  xagg[feat, dst] = sum_r xs_r^T @ S_r  over the 80
src ranks of 128.  Each rank's row of the stream tensor packs
[S (1280 dst) | x_hi (128) | x_lo (128)] in fp8, so one HBM stream on the
sync+scalar HWDGE queues feeds everything in arrival order.  The
TensorEngine consumes rank pairs with fp8 DoubleRow matmuls (k=256 per
pass, three 512/512/256-col PSUM chains); x is split hi+lo fp8 so near-
fp16 accuracy is kept (the lo correction is skipped for the last 4 rank
pairs, trading a little accuracy for time).  No SWDGE gather, no gpsimd
library swap.  The tiny evolved-weight LSTM is replicated on every core.
"""

import numpy as np

N_NODES = 10000
N_EDGES = 320000
IN_DIM = 128
HID = 256
M = 8                    # NeuronCores
NP = 10240               # padded node count (mult of 128)
RANKS = NP // 128        # 80 src ranks
NPAIR = RANKS // 2       # 40 DoubleRow rank pairs
NPC = NP // M            # 1280 dsts per core
NGRP = NPC // 128        # 10 dst groups of 128 per core
RW = NPC + 2 * 128       # stream cols per rank: S | x_hi | x_lo (chunk-blocked)
CHUNKS = [2, 2] + [4] * 19           # ranks per stream DMA (sum = 80)
CSTART = [sum(CHUNKS[:i]) for i in range(len(CHUNKS))]
GATES_AFTER = 12         # rank PAIRS before LSTM gate matmuls
DROP_LO = frozenset(range(36, 40))   # pairs whose x_lo correction is skipped

_cache = {}


def _build_module():
    import bisect
    import concourse.bacc as bacc
    import concourse.mybir as mybir
    import concourse.tile as tile

    nc = bacc.Bacc("TRN2", target_bir_lowering=False, debug=False,
                   num_devices=M)
    f32, f16, f8 = mybir.dt.float32, mybir.dt.float16, mybir.dt.float8e4
    DR = mybir.MatmulPerfMode.DoubleRow
    DRSI = mybir.MatmulPerfMode.DoubleRowSwInterleave

    # ---- DRAM inputs ----
    st_in = nc.dram_tensor("stream", [128, RANKS * RW], f8, kind="ExternalInput").ap()
    wsum_in = nc.dram_tensor("wsumT", [256, 1024], f16, kind="ExternalInput").ap()
    bsum_in = nc.dram_tensor("bsum", [1, 1024], f16, kind="ExternalInput").ap()
    iw_in = nc.dram_tensor("IW", [256, 256], f32, kind="ExternalInput").ap()
    iwt_in = nc.dram_tensor("IWT", [256, 256], f16, kind="ExternalInput").ap()
    wpt_in = nc.dram_tensor("WpT", [256, 128], f16, kind="ExternalInput").ap()
    bp_in = nc.dram_tensor("bp_col", [256, 1], f16, kind="ExternalInput").ap()
    bgcn_in = nc.dram_tensor("b_gcn", [1, 256], f16, kind="ExternalInput").ap()
    ones_in = nc.dram_tensor("ones_row", [1, 128], f16, kind="ExternalInput").ap()
    sd_in = nc.dram_tensor("sd_rows", [2, NPC], f16, kind="ExternalInput").ap()
    dcol_in = nc.dram_tensor("dinv_col", [128, NGRP], f32, kind="ExternalInput").ap()

    out_t = nc.dram_tensor("out", [NPC, HID], f16, kind="ExternalOutput").ap()

    Sig = mybir.ActivationFunctionType.Sigmoid
    Tanh = mybir.ActivationFunctionType.Tanh

    with tile.TileContext(nc) as tc:
        with (
            tc.tile_pool(name="persist", bufs=1) as pp,
            tc.tile_pool(name="schunks", bufs=len(CHUNKS)) as spool,
            tc.tile_pool(name="stage", bufs=1) as stpool,
            tc.tile_pool(name="fin", bufs=4) as fpool,
            tc.tile_pool(name="psacc_a", bufs=1, space="PSUM") as psacc_a,
            tc.tile_pool(name="psacc_b", bufs=1, space="PSUM") as psacc_b,
            tc.tile_pool(name="psacc_c", bufs=1, space="PSUM") as psacc_c,
            tc.tile_pool(name="psg", bufs=2, space="PSUM") as psg,
            tc.tile_pool(name="psl", bufs=1, space="PSUM") as psl,
        ):
            # ---------- input DMAs ----------
            # sync / scalar: stream chunks alternating (sync also takes the
            # LSTM gate weights early and the out writes late)
            # gpsimd: remaining small tensors
            wsum = pp.tile([128, 2, 1024], f16)
            iwt = pp.tile([128, 2, 256], f16)
            iw = pp.tile([128, 2, 256], f32)
            wpt = pp.tile([128, 2, 128], f16)
            bp_c = pp.tile([128, 2, 1], f16)
            bsum = pp.tile([1, 1024], f16)
            ones = pp.tile([1, 128], f16)
            # bb rows: bp@W (computed later), b_gcn (DMA'd)
            bb = pp.tile([2, 256], f16)
            sd = pp.tile([2, NPC], f16)          # rows: s2, 1/dinv
            dcol = pp.tile([128, NGRP], f32)

            schunks = [None] * len(CHUNKS)

            def emit_s_chunk(k, eng):
                sz = CHUNKS[k]
                r0 = CSTART[k]
                sch = spool.tile([128, sz * RW], f8, tag="schunk",
                                 name=f"schunk{k}")
                eng.dma_start(out=sch[:], in_=st_in[:, r0 * RW:(r0 + sz) * RW])
                schunks[k] = sch

            emit_s_chunk(0, nc.sync)
            emit_s_chunk(1, nc.scalar)
            emit_s_chunk(2, nc.sync)
            emit_s_chunk(3, nc.scalar)
            for t_, src_ in ((iwt, iwt_in), (wsum, wsum_in)):
                nc.sync.dma_start(
                    out=t_[:], in_=src_.rearrange("(k p) c -> p k c", p=128))
            nc.sync.dma_start(out=bsum[:], in_=bsum_in[:])
            for i, k in enumerate(range(4, len(CHUNKS))):
                emit_s_chunk(k, nc.scalar if i % 2 == 0 else nc.sync)

            for t_, src_ in ((iw, iw_in), (wpt, wpt_in), (bp_c, bp_in)):
                nc.gpsimd.dma_start(
                    out=t_[:], in_=src_.rearrange("(k p) c -> p k c", p=128))
            for t_, src_ in ((ones, ones_in), (sd, sd_in), (dcol, dcol_in)):
                nc.gpsimd.dma_start(out=t_[:], in_=src_[:])
            nc.gpsimd.dma_start(out=bb[1:2, :], in_=bgcn_in[:])

            # ---------- LSTM weight evolution (emitted mid rank loop) -------
            w_ev = pp.tile([128, 2, 256], f16)   # evolved GCN weight W

            def emit_gates(ic):
                gpsum = psl.tile([128, 1024], f32, space="PSUM", tag="gates",
                                 name=f"gates{ic}")
                for h in range(2):
                    gs = slice(512 * h, 512 * (h + 1))
                    nc.tensor.matmul(out=gpsum[:, gs], lhsT=ones[:, :],
                                     rhs=bsum[:, gs], start=True, stop=False)
                    nc.tensor.matmul(out=gpsum[:, gs],
                                     lhsT=iwt[:, 0, 128 * ic:128 * (ic + 1)],
                                     rhs=wsum[:, 0, gs], start=False, stop=False)
                    nc.tensor.matmul(out=gpsum[:, gs],
                                     lhsT=iwt[:, 1, 128 * ic:128 * (ic + 1)],
                                     rhs=wsum[:, 1, gs], start=False, stop=True)
                return gpsum

            def emit_lstm_post(gpsum, ic):
                si = stpool.tile([128, 256], f32, tag="si", name=f"si{ic}")
                sf = stpool.tile([128, 256], f32, tag="sf", name=f"sf{ic}")
                so = stpool.tile([128, 256], f32, tag="so", name=f"so{ic}")
                tg = stpool.tile([128, 256], f32, tag="tg", name=f"tg{ic}")
                nc.scalar.activation(out=si[:], in_=gpsum[:, 0:256], func=Sig)
                nc.scalar.activation(out=sf[:], in_=gpsum[:, 256:512], func=Sig)
                nc.scalar.activation(out=so[:], in_=gpsum[:, 768:1024], func=Sig)
                nc.scalar.activation(out=tg[:], in_=gpsum[:, 512:768], func=Tanh)
                c1 = stpool.tile([128, 256], f32, tag="c1", name=f"c1_{ic}")
                nc.vector.tensor_tensor(out=c1[:], in0=sf[:], in1=iw[:, ic, :],
                                        op=mybir.AluOpType.mult)
                c2 = stpool.tile([128, 256], f32, tag="c2", name=f"c2_{ic}")
                nc.vector.tensor_tensor(out=c2[:], in0=si[:], in1=tg[:],
                                        op=mybir.AluOpType.mult)
                cc = stpool.tile([128, 256], f32, tag="cc", name=f"cc{ic}")
                nc.vector.tensor_tensor(out=cc[:], in0=c1[:], in1=c2[:],
                                        op=mybir.AluOpType.add)
                tcc = stpool.tile([128, 256], f32, tag="tcc", name=f"tcc{ic}")
                nc.scalar.activation(out=tcc[:], in_=cc[:], func=Tanh)
                nc.vector.tensor_tensor(out=w_ev[:, ic, :], in0=so[:],
                                        in1=tcc[:], op=mybir.AluOpType.mult)

            def emit_wpw():
                wpw = pp.tile([128, 256], f16)       # Wp @ W
                wp_ps = psg.tile([128, HID], f32, space="PSUM", tag="ops",
                                 name="wp_ps")
                nc.tensor.matmul(out=wp_ps[:], lhsT=wpt[:, 0, :], rhs=w_ev[:, 0, :],
                                 start=True, stop=False)
                nc.tensor.matmul(out=wp_ps[:], lhsT=wpt[:, 1, :], rhs=w_ev[:, 1, :],
                                 start=False, stop=True)
                nc.vector.tensor_copy(out=wpw[:], in_=wp_ps[:])
                bp_ps = psg.tile([128, HID], f32, space="PSUM", tag="ops",
                                 name="bp_ps")
                nc.tensor.matmul(out=bp_ps[0:1, :], lhsT=bp_c[:, 0, :],
                                 rhs=w_ev[:, 0, :], start=True, stop=False)
                nc.tensor.matmul(out=bp_ps[0:1, :], lhsT=bp_c[:, 1, :],
                                 rhs=w_ev[:, 1, :], start=False, stop=True)
                nc.vector.tensor_copy(out=bb[0:1, :], in_=bp_ps[0:1, :])
                return wpw

            def emit_final(g, accs, wpw):
                xagg = fpool.tile([128, 128], f16, tag="xagg", name=f"xagg{g}")
                ti = 0 if g < 4 else (1 if g < 8 else 2)
                acc, gg = accs[ti], g - (0, 4, 8)[ti]
                nc.vector.tensor_copy(
                    out=xagg[:], in_=acc[:, 128 * gg:128 * (gg + 1)])
                ops = psg.tile([128, HID], f32, space="PSUM", tag="ops",
                               name=f"ops{g}")
                ds = slice(128 * g, 128 * (g + 1))
                nc.tensor.matmul(out=ops[:], lhsT=sd[:, ds], rhs=bb[:],
                                 start=True, stop=False)
                nc.tensor.matmul(out=ops[:], lhsT=xagg[:], rhs=wpw[:],
                                 start=False, stop=True)
                orow = fpool.tile([128, HID], f16, tag="orow", name=f"orow{g}")
                nc.scalar.activation(out=orow[:], in_=ops[:],
                                     func=mybir.ActivationFunctionType.Copy,
                                     scale=dcol[:, g:g + 1])
                nc.sync.dma_start(
                    out=out_t.rearrange("(g p) h -> g p h", p=128)[g],
                    in_=orow[:],
                )

            # ---------- main: DoubleRow pass over 40 src rank pairs ---------
            acc_a = psacc_a.tile([128, 512], f32, space="PSUM", tag="acc_a")
            acc_b = psacc_b.tile([128, 512], f32, space="PSUM", tag="acc_b")
            acc_c = psacc_c.tile([128, 256], f32, space="PSUM", tag="acc_c")
            TILES = [(0, 512, acc_a), (512, 512, acc_b), (1024, 256, acc_c)]
            gp0 = gp1 = None
            wpw = None
            for pr in range(NPAIR):
                if pr == GATES_AFTER:
                    gp0 = emit_gates(0)
                if pr == GATES_AFTER + 8:
                    emit_lstm_post(gp0, 0)
                    gp1 = emit_gates(1)
                if pr == GATES_AFTER + 16:
                    emit_lstm_post(gp1, 1)
                if pr == GATES_AFTER + 20:
                    wpw = emit_wpw()
                r = 2 * pr
                k = bisect.bisect_right(CSTART, r) - 1
                jj = r - CSTART[k]
                sz = CHUNKS[k]
                sch = schunks[k]
                srow = sch[:, 0:sz * NPC].rearrange("p (j c) -> p j c", c=NPC)
                last = pr == NPAIR - 1
                last_hl = 0 if NPAIR - 1 in DROP_LO else 1
                for hl in range(2):
                    if hl == 1 and pr in DROP_LO:
                        continue
                    xb = sz * NPC + hl * sz * 128
                    x_t = sch[:, xb + jj * 128:xb + (jj + 2) * 128] \
                        .rearrange("p (j c) -> p j c", c=128)
                    for ti, (c0, w, acc) in enumerate(TILES):
                        nc.tensor.matmul(
                            out=acc[:],
                            lhsT=x_t,
                            rhs=srow[:, jj:jj + 2, c0:c0 + w],
                            start=(pr == 0 and hl == 0),
                            stop=(last and hl == last_hl),
                            perf_mode=DR,
                        )

            for g in range(NGRP):
                emit_final(g, (acc_a, acc_b, acc_c), wpw)

    nc.compile()
    return nc


def _preprocess(edge_index):
    """Host-side: degree norms, per-core fp8 count matrices (self loops in)."""
    src = np.asarray(edge_index[0], dtype=np.int64)
    dst = np.asarray(edge_index[1], dtype=np.int64)
    deg = np.bincount(dst, minlength=N_NODES).astype(np.float64) + 1.0
    dinv = (1.0 / np.sqrt(deg)).astype(np.float32)

    # s2[d] = sum over in-edges of dinv[src], self loop included
    s2 = (np.bincount(dst, weights=dinv[src].astype(np.float64),
                      minlength=N_NODES) + dinv.astype(np.float64)).astype(np.float32)
    return dinv, s2, src, dst


LAST_RESULT = None


def kernel(x, edge_index, Wp, bp, W_ih, W_hh, b_ih, b_hh, initial_weight, b_gcn):
    global LAST_RESULT
    import ml_dtypes
    from concourse.bass_utils import run_bass_kernel_spmd

    f8 = ml_dtypes.float8_e4m3
    x = np.asarray(x, np.float32)
    Wp = np.asarray(Wp, np.float32)
    bp = np.asarray(bp, np.float32)
    W_ih = np.asarray(W_ih, np.float32)
    W_hh = np.asarray(W_hh, np.float32)
    b_ih = np.asarray(b_ih, np.float32)
    b_hh = np.asarray(b_hh, np.float32)
    initial_weight = np.asarray(initial_weight, np.float32)
    b_gcn = np.asarray(b_gcn, np.float32)
    assert x.shape == (N_NODES, IN_DIM)

    dinv, s2, src, dst = _preprocess(edge_index)

    if "nc" not in _cache:
        _cache["nc"] = _build_module()
    nc = _cache["nc"]

    # host pre-scales x rows by dinv[src]; hi+lo fp8 token tables
    xp = np.zeros((NP, IN_DIM), np.float32)
    xp[:N_NODES] = x * dinv[:, None]
    xp_t = np.ascontiguousarray(
        xp.reshape(RANKS, 128, IN_DIM).transpose(1, 0, 2))  # [128, R, 128]
    xhi = xp_t.astype(f8)
    xlo = (xp_t - xhi.astype(np.float32)).astype(f8)

    def swi(xq):
        # DoubleRowSwInterleave weight layout per rank pair (A, B):
        # flat cols [A_127, B_127, A_126, B_126, ..., A_0, B_0]
        pq = xq.reshape(128, NPAIR, 2, 128)
        o = np.empty((128, NPAIR, 256), xq.dtype)
        o[:, :, 0::2] = pq[:, :, 0, ::-1]
        o[:, :, 1::2] = pq[:, :, 1, ::-1]
        return o

    xhi_swi = swi(xhi)
    xlo_swi = swi(xlo)

    core = dst // NPC
    dloc = dst - core * NPC

    wsumT = np.ascontiguousarray((W_ih + W_hh).T).astype(np.float16)
    bsum = (b_ih + b_hh).reshape(1, -1).astype(np.float16)

    s2p = np.zeros(NP, np.float32)
    s2p[:N_NODES] = s2
    drip = np.zeros(NP, np.float32)
    drip[:N_NODES] = 1.0 / dinv
    dlocp = np.zeros(NP, np.float32)
    dlocp[:N_NODES] = dinv

    shared = {
        "wsumT": wsumT,
        "bsum": bsum,
        "IW": initial_weight,
        "IWT": np.ascontiguousarray(initial_weight.T).astype(np.float16),
        "WpT": np.ascontiguousarray(Wp.T).astype(np.float16),
        "bp_col": np.ascontiguousarray(bp.reshape(-1, 1)).astype(np.float16),
        "b_gcn": b_gcn.reshape(1, -1).astype(np.float16),
        "ones_row": np.ones((1, 128), np.float16),
    }
    in_maps = []
    for c in range(M):
        m = core == c
        flat = src[m] * NPC + dloc[m]
        cnt = np.bincount(flat, minlength=NP * NPC)
        d0, d1 = c * NPC, min((c + 1) * NPC, N_NODES)
        dd = np.arange(d0, d1, dtype=np.int64)
        cnt[dd * NPC + (dd - d0)] += 1     # self loops
        sc = cnt.reshape(RANKS, 128, NPC).transpose(1, 0, 2)  # [128, R, NPC]
        sc8 = sc.astype(f8)
        stream = np.empty((128, RANKS * RW), f8)
        for k, sz in enumerate(CHUNKS):
            r0 = CSTART[k]
            base = r0 * RW
            ns = sz * NPC
            stream[:, base:base + ns] = sc8[:, r0:r0 + sz, :].reshape(128, ns)
            stream[:, base + ns:base + ns + sz * 128] = \
                xhi[:, r0:r0 + sz, :].reshape(128, sz * 128)
            stream[:, base + ns + sz * 128:base + sz * RW] = \
                xlo[:, r0:r0 + sz, :].reshape(128, sz * 128)
        sl = slice(c * NPC, (c + 1) * NPC)
        in_maps.append({
            **shared,
            "stream": stream,
            "sd_rows": np.stack([s2p[sl], drip[sl]]).astype(np.float16),
            "dinv_col": np.ascontiguousarray(
                dlocp[sl].reshape(NGRP, 128).T),
        })

    res = run_bass_kernel_spmd(nc, in_maps, list(range(M)))
    LAST_RESULT = res

    out = np.empty((N_NODES, HID), np.float32)
    for c in range(M):
        d0, d1 = c * NPC, min((c + 1) * NPC, N_NODES)
        out[d0:d1] = res.results[c]["out"][:d1 - d0].astype(np.float32)
    return out


# revision 27
# speedup vs baseline: 1.0156x; 1.0140x over previous
"""Trainium2 Bass kernel for EvolveGCN-O forward (GCN message passing).

Math (reference):
    h   = x @ Wp + bp
    W   = LSTM-evolved weight from initial_weight (one step, h0=c0=IW)
    hw  = h @ W
    out = D^-1/2 (A+I) D^-1/2 hw + b_gcn

Factored for the kernel:
    out[d] = dinv[d] * (sum_{e: dst=d} dinv[src_e] * x[src_e]) @ (Wp @ W)
             + s2[d]*dinv[d]*(bp @ W) + b_gcn
with s2[d] = sum_{e in(d)} dinv[src_e] (self loops included as edges).

Distribution: dst nodes sharded contiguously over 8 NeuronCores (1280 each).
The aggregation sum_{e:dst=d} dinv[src]*x[src] is computed as dense matmuls
against a per-core count matrix S[src, dst_local] (fp8, exact small ints;
self loops folded in):  xagg[feat, dst] = sum_r xs_r^T @ S_r  over the 80
src ranks of 128.  Each rank's row of the stream tensor packs
[S (1280 dst) | x_hi (128) | x_lo (128)] in fp8, so one HBM stream on the
sync+scalar HWDGE queues feeds everything in arrival order.  The
TensorEngine consumes rank pairs with fp8 DoubleRow matmuls (k=256 per
pass, three 512/512/256-col PSUM chains); x is split hi+lo fp8 so near-
fp16 accuracy is kept (the lo correction is skipped for the last 4 rank
pairs, trading a little accuracy for time).  No SWDGE gather, no gpsimd
library swap.  The tiny evolved-weight LSTM is replicated on every core.
"""

import numpy as np

N_NODES = 10000
N_EDGES = 320000
IN_DIM = 128
HID = 256
M = 8                    # NeuronCores
NP = 10240               # padded node count (mult of 128)
RANKS = NP // 128        # 80 src ranks
NPAIR = RANKS // 2       # 40 DoubleRow rank pairs
NPC = NP // M            # 1280 dsts per core
NGRP = NPC // 128        # 10 dst groups of 128 per core
RW = NPC + 2 * 128       # stream cols per rank: S | x_hi | x_lo (chunk-blocked)
CHUNKS = [2] * 10 + [4] * 15         # ranks per stream DMA (sum = 80)
CSTART = [sum(CHUNKS[:i]) for i in range(len(CHUNKS))]
GATES_AFTER = 12         # rank PAIRS before LSTM gate matmuls
DROP_LO = frozenset(range(36, 40))   # pairs whose x_lo correction is skipped

_cache = {}


def _build_module():
    import bisect
    import concourse.bacc as bacc
    import concourse.mybir as mybir
    import concourse.tile as tile

    nc = bacc.Bacc("TRN2", target_bir_lowering=False, debug=False,
                   num_devices=M)
    f32, f16, f8 = mybir.dt.float32, mybir.dt.float16, mybir.dt.float8e4
    DR = mybir.MatmulPerfMode.DoubleRow
    DRSI = mybir.MatmulPerfMode.DoubleRowSwInterleave

    # ---- DRAM inputs ----
    st_in = nc.dram_tensor("stream", [128, RANKS * RW], f8, kind="ExternalInput").ap()
    wsum_in = nc.dram_tensor("wsumT", [256, 1024], f16, kind="ExternalInput").ap()
    bsum_in = nc.dram_tensor("bsum", [1, 1024], f16, kind="ExternalInput").ap()
    iw_in = nc.dram_tensor("IW", [256, 256], f32, kind="ExternalInput").ap()
    iwt_in = nc.dram_tensor("IWT", [256, 256], f16, kind="ExternalInput").ap()
    wpt_in = nc.dram_tensor("WpT", [256, 128], f16, kind="ExternalInput").ap()
    bp_in = nc.dram_tensor("bp_col", [256, 1], f16, kind="ExternalInput").ap()
    bgcn_in = nc.dram_tensor("b_gcn", [1, 256], f16, kind="ExternalInput").ap()
    ones_in = nc.dram_tensor("ones_row", [1, 128], f16, kind="ExternalInput").ap()
    sd_in = nc.dram_tensor("sd_rows", [2, NPC], f16, kind="ExternalInput").ap()
    dcol_in = nc.dram_tensor("dinv_col", [128, NGRP], f32, kind="ExternalInput").ap()

    out_t = nc.dram_tensor("out", [NPC, HID], f16, kind="ExternalOutput").ap()

    Sig = mybir.ActivationFunctionType.Sigmoid
    Tanh = mybir.ActivationFunctionType.Tanh

    with tile.TileContext(nc) as tc:
        with (
            tc.tile_pool(name="persist", bufs=1) as pp,
            tc.tile_pool(name="schunks", bufs=len(CHUNKS)) as spool,
            tc.tile_pool(name="stage", bufs=1) as stpool,
            tc.tile_pool(name="fin", bufs=4) as fpool,
            tc.tile_pool(name="psacc_a", bufs=1, space="PSUM") as psacc_a,
            tc.tile_pool(name="psacc_b", bufs=1, space="PSUM") as psacc_b,
            tc.tile_pool(name="psacc_c", bufs=1, space="PSUM") as psacc_c,
            tc.tile_pool(name="psg", bufs=2, space="PSUM") as psg,
            tc.tile_pool(name="psl", bufs=1, space="PSUM") as psl,
        ):
            # ---------- input DMAs ----------
            # sync / scalar: stream chunks alternating (sync also takes the
            # LSTM gate weights early and the out writes late)
            # gpsimd: remaining small tensors
            wsum = pp.tile([128, 2, 1024], f16)
            iwt = pp.tile([128, 2, 256], f16)
            iw = pp.tile([128, 2, 256], f32)
            wpt = pp.tile([128, 2, 128], f16)
            bp_c = pp.tile([128, 2, 1], f16)
            bsum = pp.tile([1, 1024], f16)
            ones = pp.tile([1, 128], f16)
            # bb rows: bp@W (computed later), b_gcn (DMA'd)
            bb = pp.tile([2, 256], f16)
            sd = pp.tile([2, NPC], f16)          # rows: s2, 1/dinv
            dcol = pp.tile([128, NGRP], f32)

            schunks = [None] * len(CHUNKS)

            def emit_s_chunk(k, eng):
                sz = CHUNKS[k]
                r0 = CSTART[k]
                sch = spool.tile([128, sz * RW], f8, tag="schunk",
                                 name=f"schunk{k}")
                eng.dma_start(out=sch[:], in_=st_in[:, r0 * RW:(r0 + sz) * RW])
                schunks[k] = sch

            emit_s_chunk(0, nc.sync)
            emit_s_chunk(1, nc.scalar)
            emit_s_chunk(2, nc.sync)
            emit_s_chunk(3, nc.scalar)
            for t_, src_ in ((iwt, iwt_in), (wsum, wsum_in)):
                nc.sync.dma_start(
                    out=t_[:], in_=src_.rearrange("(k p) c -> p k c", p=128))
            nc.sync.dma_start(out=bsum[:], in_=bsum_in[:])
            for i, k in enumerate(range(4, len(CHUNKS))):
                emit_s_chunk(k, nc.scalar if i % 2 == 0 else nc.sync)

            for t_, src_ in ((iw, iw_in), (wpt, wpt_in), (bp_c, bp_in)):
                nc.gpsimd.dma_start(
                    out=t_[:], in_=src_.rearrange("(k p) c -> p k c", p=128))
            for t_, src_ in ((ones, ones_in), (sd, sd_in), (dcol, dcol_in)):
                nc.gpsimd.dma_start(out=t_[:], in_=src_[:])
            nc.gpsimd.dma_start(out=bb[1:2, :], in_=bgcn_in[:])

            # ---------- LSTM weight evolution (emitted mid rank loop) -------
            w_ev = pp.tile([128, 2, 256], f16)   # evolved GCN weight W

            def emit_gates(ic):
                gpsum = psl.tile([128, 1024], f32, space="PSUM", tag="gates",
                                 name=f"gates{ic}")
                for h in range(2):
                    gs = slice(512 * h, 512 * (h + 1))
                    nc.tensor.matmul(out=gpsum[:, gs], lhsT=ones[:, :],
                                     rhs=bsum[:, gs], start=True, stop=False)
                    nc.tensor.matmul(out=gpsum[:, gs],
                                     lhsT=iwt[:, 0, 128 * ic:128 * (ic + 1)],
                                     rhs=wsum[:, 0, gs], start=False, stop=False)
                    nc.tensor.matmul(out=gpsum[:, gs],
                                     lhsT=iwt[:, 1, 128 * ic:128 * (ic + 1)],
                                     rhs=wsum[:, 1, gs], start=False, stop=True)
                return gpsum

            def emit_lstm_post(gpsum, ic):
                si = stpool.tile([128, 256], f32, tag="si", name=f"si{ic}")
                sf = stpool.tile([128, 256], f32, tag="sf", name=f"sf{ic}")
                so = stpool.tile([128, 256], f32, tag="so", name=f"so{ic}")
                tg = stpool.tile([128, 256], f32, tag="tg", name=f"tg{ic}")
                nc.scalar.activation(out=si[:], in_=gpsum[:, 0:256], func=Sig)
                nc.scalar.activation(out=sf[:], in_=gpsum[:, 256:512], func=Sig)
                nc.scalar.activation(out=so[:], in_=gpsum[:, 768:1024], func=Sig)
                nc.scalar.activation(out=tg[:], in_=gpsum[:, 512:768], func=Tanh)
                c1 = stpool.tile([128, 256], f32, tag="c1", name=f"c1_{ic}")
                nc.vector.tensor_tensor(out=c1[:], in0=sf[:], in1=iw[:, ic, :],
                                        op=mybir.AluOpType.mult)
                c2 = stpool.tile([128, 256], f32, tag="c2", name=f"c2_{ic}")
                nc.vector.tensor_tensor(out=c2[:], in0=si[:], in1=tg[:],
                                        op=mybir.AluOpType.mult)
                cc = stpool.tile([128, 256], f32, tag="cc", name=f"cc{ic}")
                nc.vector.tensor_tensor(out=cc[:], in0=c1[:], in1=c2[:],
                                        op=mybir.AluOpType.add)
                tcc = stpool.tile([128, 256], f32, tag="tcc", name=f"tcc{ic}")
                nc.scalar.activation(out=tcc[:], in_=cc[:], func=Tanh)
                nc.vector.tensor_tensor(out=w_ev[:, ic, :], in0=so[:],
                                        in1=tcc[:], op=mybir.AluOpType.mult)

            def emit_wpw():
                wpw = pp.tile([128, 256], f16)       # Wp @ W
                wp_ps = psg.tile([128, HID], f32, space="PSUM", tag="ops",
                                 name="wp_ps")
                nc.tensor.matmul(out=wp_ps[:], lhsT=wpt[:, 0, :], rhs=w_ev[:, 0, :],
                                 start=True, stop=False)
                nc.tensor.matmul(out=wp_ps[:], lhsT=wpt[:, 1, :], rhs=w_ev[:, 1, :],
                                 start=False, stop=True)
                nc.vector.tensor_copy(out=wpw[:], in_=wp_ps[:])
                bp_ps = psg.tile([128, HID], f32, space="PSUM", tag="ops",
                                 name="bp_ps")
                nc.tensor.matmul(out=bp_ps[0:1, :], lhsT=bp_c[:, 0, :],
                                 rhs=w_ev[:, 0, :], start=True, stop=False)
                nc.tensor.matmul(out=bp_ps[0:1, :], lhsT=bp_c[:, 1, :],
                                 rhs=w_ev[:, 1, :], start=False, stop=True)
                nc.vector.tensor_copy(out=bb[0:1, :], in_=bp_ps[0:1, :])
                return wpw

            def emit_final(g, accs, wpw):
                xagg = fpool.tile([128, 128], f16, tag="xagg", name=f"xagg{g}")
                ti = 0 if g < 4 else (1 if g < 8 else 2)
                acc, gg = accs[ti], g - (0, 4, 8)[ti]
                nc.vector.tensor_copy(
                    out=xagg[:], in_=acc[:, 128 * gg:128 * (gg + 1)])
                ops = psg.tile([128, HID], f32, space="PSUM", tag="ops",
                               name=f"ops{g}")
                ds = slice(128 * g, 128 * (g + 1))
                nc.tensor.matmul(out=ops[:], lhsT=sd[:, ds], rhs=bb[:],
                                 start=True, stop=False)
                nc.tensor.matmul(out=ops[:], lhsT=xagg[:], rhs=wpw[:],
                                 start=False, stop=True)
                orow = fpool.tile([128, HID], f16, tag="orow", name=f"orow{g}")
                nc.scalar.activation(out=orow[:], in_=ops[:],
                                     func=mybir.ActivationFunctionType.Copy,
                                     scale=dcol[:, g:g + 1])
                nc.sync.dma_start(
                    out=out_t.rearrange("(g p) h -> g p h", p=128)[g],
                    in_=orow[:],
                )

            # ---------- main: DoubleRow pass over 40 src rank pairs ---------
            acc_a = psacc_a.tile([128, 512], f32, space="PSUM", tag="acc_a")
            acc_b = psacc_b.tile([128, 512], f32, space="PSUM", tag="acc_b")
            acc_c = psacc_c.tile([128, 256], f32, space="PSUM", tag="acc_c")
            TILES = [(0, 512, acc_a), (512, 512, acc_b), (1024, 256, acc_c)]
            gp0 = gp1 = None
            wpw = None
            for pr in range(NPAIR):
                if pr == GATES_AFTER:
                    gp0 = emit_gates(0)
                if pr == GATES_AFTER + 8:
                    emit_lstm_post(gp0, 0)
                    gp1 = emit_gates(1)
                if pr == GATES_AFTER + 16:
                    emit_lstm_post(gp1, 1)
                if pr == GATES_AFTER + 20:
                    wpw = emit_wpw()
                r = 2 * pr
                k = bisect.bisect_right(CSTART, r) - 1
                jj = r - CSTART[k]
                sz = CHUNKS[k]
                sch = schunks[k]
                srow = sch[:, 0:sz * NPC].rearrange("p (j c) -> p j c", c=NPC)
                last = pr == NPAIR - 1
                last_hl = 0 if NPAIR - 1 in DROP_LO else 1
                for hl in range(2):
                    if hl == 1 and pr in DROP_LO:
                        continue
                    xb = sz * NPC + hl * sz * 128
                    x_t = sch[:, xb + jj * 128:xb + (jj + 2) * 128] \
                        .rearrange("p (j c) -> p j c", c=128)
                    for ti, (c0, w, acc) in enumerate(TILES):
                        nc.tensor.matmul(
                            out=acc[:],
                            lhsT=x_t,
                            rhs=srow[:, jj:jj + 2, c0:c0 + w],
                            start=(pr == 0 and hl == 0),
                            stop=(last and hl == last_hl),
                            perf_mode=DR,
                        )

            for g in range(NGRP):
                emit_final(g, (acc_a, acc_b, acc_c), wpw)

    nc.compile()
    return nc


def _preprocess(edge_index):
    """Host-side: degree norms, per-core fp8 count matrices (self loops in)."""
    src = np.asarray(edge_index[0], dtype=np.int64)
    dst = np.asarray(edge_index[1], dtype=np.int64)
    deg = np.bincount(dst, minlength=N_NODES).astype(np.float64) + 1.0
    dinv = (1.0 / np.sqrt(deg)).astype(np.float32)

    # s2[d] = sum over in-edges of dinv[src], self loop included
    s2 = (np.bincount(dst, weights=dinv[src].astype(np.float64),
                      minlength=N_NODES) + dinv.astype(np.float64)).astype(np.float32)
    return dinv, s2, src, dst


LAST_RESULT = None


def kernel(x, edge_index, Wp, bp, W_ih, W_hh, b_ih, b_hh, initial_weight, b_gcn):
    global LAST_RESULT
    import ml_dtypes
    from concourse.bass_utils import run_bass_kernel_spmd

    f8 = ml_dtypes.float8_e4m3
    x = np.asarray(x, np.float32)
    Wp = np.asarray(Wp, np.float32)
    bp = np.asarray(bp, np.float32)
    W_ih = np.asarray(W_ih, np.float32)
    W_hh = np.asarray(W_hh, np.float32)
    b_ih = np.asarray(b_ih, np.float32)
    b_hh = np.asarray(b_hh, np.float32)
    initial_weight = np.asarray(initial_weight, np.float32)
    b_gcn = np.asarray(b_gcn, np.float32)
    assert x.shape == (N_NODES, IN_DIM)

    dinv, s2, src, dst = _preprocess(edge_index)

    if "nc" not in _cache:
        _cache["nc"] = _build_module()
    nc = _cache["nc"]

    # host pre-scales x rows by dinv[src]; hi+lo fp8 token tables
    xp = np.zeros((NP, IN_DIM), np.float32)
    xp[:N_NODES] = x * dinv[:, None]
    xp_t = np.ascontiguousarray(
        xp.reshape(RANKS, 128, IN_DIM).transpose(1, 0, 2))  # [128, R, 128]
    xhi = xp_t.astype(f8)
    xlo = (xp_t - xhi.astype(np.float32)).astype(f8)

    def swi(xq):
        # DoubleRowSwInterleave weight layout per rank pair (A, B):
        # flat cols [A_127, B_127, A_126, B_126, ..., A_0, B_0]
        pq = xq.reshape(128, NPAIR, 2, 128)
        o = np.empty((128, NPAIR, 256), xq.dtype)
        o[:, :, 0::2] = pq[:, :, 0, ::-1]
        o[:, :, 1::2] = pq[:, :, 1, ::-1]
        return o

    xhi_swi = swi(xhi)
    xlo_swi = swi(xlo)

    core = dst // NPC
    dloc = dst - core * NPC

    wsumT = np.ascontiguousarray((W_ih + W_hh).T).astype(np.float16)
    bsum = (b_ih + b_hh).reshape(1, -1).astype(np.float16)

    s2p = np.zeros(NP, np.float32)
    s2p[:N_NODES] = s2
    drip = np.zeros(NP, np.float32)
    drip[:N_NODES] = 1.0 / dinv
    dlocp = np.zeros(NP, np.float32)
    dlocp[:N_NODES] = dinv

    shared = {
        "wsumT": wsumT,
        "bsum": bsum,
        "IW": initial_weight,
        "IWT": np.ascontiguousarray(initial_weight.T).astype(np.float16),
        "WpT": np.ascontiguousarray(Wp.T).astype(np.float16),
        "bp_col": np.ascontiguousarray(bp.reshape(-1, 1)).astype(np.float16),
        "b_gcn": b_gcn.reshape(1, -1).astype(np.float16),
        "ones_row": np.ones((1, 128), np.float16),
    }
    in_maps = []
    for c in range(M):
        m = core == c
        flat = src[m] * NPC + dloc[m]
        cnt = np.bincount(flat, minlength=NP * NPC)
        d0, d1 = c * NPC, min((c + 1) * NPC, N_NODES)
        dd = np.arange(d0, d1, dtype=np.int64)
        cnt[dd * NPC + (dd - d0)] += 1     # self loops
        sc = cnt.reshape(RANKS, 128, NPC).transpose(1, 0, 2)  # [128, R, NPC]
        sc8 = sc.astype(f8)
        stream = np.empty((128, RANKS * RW), f8)
        for k, sz in enumerate(CHUNKS):
            r0 = CSTART[k]
            base = r0 * RW
            ns = sz * NPC
            stream[:, base:base + ns] = sc8[:, r0:r0 + sz, :].reshape(128, ns)
            stream[:, base + ns:base + ns + sz * 128] = \
                xhi[:, r0:r0 + sz, :].reshape(128, sz * 128)
            stream[:, base + ns + sz * 128:base + sz * RW] = \
                xlo[:, r0:r0 + sz, :].reshape(128, sz * 128)
        sl = slice(c * NPC, (c + 1) * NPC)
        in_maps.append({
            **shared,
            "stream": stream,
            "sd_rows": np.stack([s2p[sl], drip[sl]]).astype(np.float16),
            "dinv_col": np.ascontiguousarray(
                dlocp[sl].reshape(NGRP, 128).T),
        })

    res = run_bass_kernel_spmd(nc, in_maps, list(range(M)))
    LAST_RESULT = res

    out = np.empty((N_NODES, HID), np.float32)
    for c in range(M):
        d0, d1 = c * NPC, min((c + 1) * NPC, N_NODES)
        out[d0:d1] = res.results[c]["out"][:d1 - d0].astype(np.float32)
    return out


# revision 28
# speedup vs baseline: 1.0562x; 1.0399x over previous
"""Trainium2 Bass kernel for EvolveGCN-O forward (GCN message passing).

Math (reference):
    h   = x @ Wp + bp
    W   = LSTM-evolved weight from initial_weight (one step, h0=c0=IW)
    hw  = h @ W
    out = D^-1/2 (A+I) D^-1/2 hw + b_gcn

Factored for the kernel:
    out[d] = dinv[d] * (sum_{e: dst=d} dinv[src_e] * x[src_e]) @ (Wp @ W)
             + s2[d]*dinv[d]*(bp @ W) + b_gcn
with s2[d] = sum_{e in(d)} dinv[src_e] (self loops included as edges).

Distribution: dst nodes sharded contiguously over 8 NeuronCores (1280 each).
The aggregation sum_{e:dst=d} dinv[src]*x[src] is computed as dense matmuls
against a per-core count matrix S[src, dst_local] (fp8, exact small ints;
self loops folded in):  xagg[feat, dst] = sum_r xs_r^T @ S_r  over the 80
src ranks of 128.  Each rank's row of the stream tensor packs
[S (1280 dst) | x_hi (128) | x_lo (128)] in fp8, so one HBM stream on the
sync+scalar HWDGE queues feeds everything in arrival order.  The
TensorEngine consumes rank pairs with fp8 DoubleRow matmuls (k=256 per
pass, three 512/512/256-col PSUM chains); x is split hi+lo fp8 so near-
fp16 accuracy is kept (the lo correction is skipped for the last 4 rank
pairs, trading a little accuracy for time).  No SWDGE gather, no gpsimd
library swap.  The tiny evolved-weight LSTM is replicated on every core.
"""

import numpy as np

N_NODES = 10000
N_EDGES = 320000
IN_DIM = 128
HID = 256
M = 8                    # NeuronCores
NP = 10240               # padded node count (mult of 128)
RANKS = NP // 128        # 80 src ranks
NPAIR = RANKS // 2       # 40 DoubleRow rank pairs
NPC = NP // M            # 1280 dsts per core
NGRP = NPC // 128        # 10 dst groups of 128 per core
RW = NPC + 2 * 128       # stream cols per rank: S | x_hi | x_lo (chunk-blocked)
CHUNKS = [2, 2] + [4] * 19           # ranks per stream DMA (sum = 80)
CSTART = [sum(CHUNKS[:i]) for i in range(len(CHUNKS))]
GATES_AFTER = 8          # rank PAIRS before LSTM gate matmuls
DROP_LO = frozenset(range(36, 40))   # pairs whose x_lo correction is skipped

_cache = {}


def _build_module():
    import bisect
    import concourse.bacc as bacc
    import concourse.mybir as mybir
    import concourse.tile as tile

    nc = bacc.Bacc("TRN2", target_bir_lowering=False, debug=False,
                   num_devices=M)
    f32, f16, f8 = mybir.dt.float32, mybir.dt.float16, mybir.dt.float8e4
    DR = mybir.MatmulPerfMode.DoubleRow
    DRSI = mybir.MatmulPerfMode.DoubleRowSwInterleave

    # ---- DRAM inputs ----
    st_in = nc.dram_tensor("stream", [128, RANKS * RW], f8, kind="ExternalInput").ap()
    wsum_in = nc.dram_tensor("wsumT", [256, 1024], f16, kind="ExternalInput").ap()
    bsum_in = nc.dram_tensor("bsum", [1, 1024], f16, kind="ExternalInput").ap()
    iw_in = nc.dram_tensor("IW", [256, 256], f32, kind="ExternalInput").ap()
    iwt_in = nc.dram_tensor("IWT", [256, 256], f16, kind="ExternalInput").ap()
    wpt_in = nc.dram_tensor("WpT", [256, 128], f16, kind="ExternalInput").ap()
    bp_in = nc.dram_tensor("bp_col", [256, 1], f16, kind="ExternalInput").ap()
    bgcn_in = nc.dram_tensor("b_gcn", [1, 256], f16, kind="ExternalInput").ap()
    ones_in = nc.dram_tensor("ones_row", [1, 128], f16, kind="ExternalInput").ap()
    sd_in = nc.dram_tensor("sd_rows", [2, NPC], f16, kind="ExternalInput").ap()
    dcol_in = nc.dram_tensor("dinv_col", [128, NGRP], f32, kind="ExternalInput").ap()

    out_t = nc.dram_tensor("out", [NPC, HID], f16, kind="ExternalOutput").ap()

    Sig = mybir.ActivationFunctionType.Sigmoid
    Tanh = mybir.ActivationFunctionType.Tanh

    with tile.TileContext(nc) as tc:
        with (
            tc.tile_pool(name="persist", bufs=1) as pp,
            tc.tile_pool(name="schunks", bufs=len(CHUNKS)) as spool,
            tc.tile_pool(name="stage", bufs=1) as stpool,
            tc.tile_pool(name="fin", bufs=4) as fpool,
            tc.tile_pool(name="psacc_a", bufs=1, space="PSUM") as psacc_a,
            tc.tile_pool(name="psacc_b", bufs=1, space="PSUM") as psacc_b,
            tc.tile_pool(name="psacc_c", bufs=1, space="PSUM") as psacc_c,
            tc.tile_pool(name="psg", bufs=2, space="PSUM") as psg,
            tc.tile_pool(name="psl", bufs=1, space="PSUM") as psl,
        ):
            # ---------- input DMAs ----------
            # sync / scalar: stream chunks alternating (sync also takes the
            # LSTM gate weights early and the out writes late)
            # gpsimd: remaining small tensors
            wsum = pp.tile([128, 2, 1024], f16)
            iwt = pp.tile([128, 2, 256], f16)
            iw = pp.tile([128, 2, 256], f32)
            wpt = pp.tile([128, 2, 128], f16)
            bp_c = pp.tile([128, 2, 1], f16)
            bsum = pp.tile([1, 1024], f16)
            ones = pp.tile([1, 128], f16)
            # bb rows: bp@W (computed later), b_gcn (DMA'd)
            bb = pp.tile([2, 256], f16)
            sd = pp.tile([2, NPC], f16)          # rows: s2, 1/dinv
            dcol = pp.tile([128, NGRP], f32)

            schunks = [None] * len(CHUNKS)

            def emit_s_chunk(k, eng):
                sz = CHUNKS[k]
                r0 = CSTART[k]
                sch = spool.tile([128, sz * RW], f8, tag="schunk",
                                 name=f"schunk{k}")
                eng.dma_start(out=sch[:], in_=st_in[:, r0 * RW:(r0 + sz) * RW])
                schunks[k] = sch

            emit_s_chunk(0, nc.sync)
            emit_s_chunk(1, nc.scalar)
            emit_s_chunk(2, nc.sync)
            emit_s_chunk(3, nc.scalar)
            for t_, src_ in ((iwt, iwt_in), (wsum, wsum_in)):
                nc.sync.dma_start(
                    out=t_[:], in_=src_.rearrange("(k p) c -> p k c", p=128))
            nc.sync.dma_start(out=bsum[:], in_=bsum_in[:])
            for i, k in enumerate(range(4, len(CHUNKS))):
                emit_s_chunk(k, nc.scalar if i % 2 == 0 else nc.sync)

            for t_, src_ in ((iw, iw_in), (wpt, wpt_in), (bp_c, bp_in)):
                nc.gpsimd.dma_start(
                    out=t_[:], in_=src_.rearrange("(k p) c -> p k c", p=128))
            for t_, src_ in ((ones, ones_in), (sd, sd_in), (dcol, dcol_in)):
                nc.gpsimd.dma_start(out=t_[:], in_=src_[:])
            nc.gpsimd.dma_start(out=bb[1:2, :], in_=bgcn_in[:])

            # ---------- LSTM weight evolution (emitted mid rank loop) -------
            w_ev = pp.tile([128, 2, 256], f16)   # evolved GCN weight W

            def emit_gates(ic):
                gpsum = psl.tile([128, 1024], f32, space="PSUM", tag="gates",
                                 name=f"gates{ic}")
                for h in range(2):
                    gs = slice(512 * h, 512 * (h + 1))
                    nc.tensor.matmul(out=gpsum[:, gs], lhsT=ones[:, :],
                                     rhs=bsum[:, gs], start=True, stop=False)
                    nc.tensor.matmul(out=gpsum[:, gs],
                                     lhsT=iwt[:, 0, 128 * ic:128 * (ic + 1)],
                                     rhs=wsum[:, 0, gs], start=False, stop=False)
                    nc.tensor.matmul(out=gpsum[:, gs],
                                     lhsT=iwt[:, 1, 128 * ic:128 * (ic + 1)],
                                     rhs=wsum[:, 1, gs], start=False, stop=True)
                return gpsum

            def emit_lstm_post(gpsum, ic):
                si = stpool.tile([128, 256], f32, tag="si", name=f"si{ic}")
                sf = stpool.tile([128, 256], f32, tag="sf", name=f"sf{ic}")
                so = stpool.tile([128, 256], f32, tag="so", name=f"so{ic}")
                tg = stpool.tile([128, 256], f32, tag="tg", name=f"tg{ic}")
                nc.scalar.activation(out=si[:], in_=gpsum[:, 0:256], func=Sig)
                nc.scalar.activation(out=sf[:], in_=gpsum[:, 256:512], func=Sig)
                nc.scalar.activation(out=so[:], in_=gpsum[:, 768:1024], func=Sig)
                nc.scalar.activation(out=tg[:], in_=gpsum[:, 512:768], func=Tanh)
                c1 = stpool.tile([128, 256], f32, tag="c1", name=f"c1_{ic}")
                nc.vector.tensor_tensor(out=c1[:], in0=sf[:], in1=iw[:, ic, :],
                                        op=mybir.AluOpType.mult)
                c2 = stpool.tile([128, 256], f32, tag="c2", name=f"c2_{ic}")
                nc.vector.tensor_tensor(out=c2[:], in0=si[:], in1=tg[:],
                                        op=mybir.AluOpType.mult)
                cc = stpool.tile([128, 256], f32, tag="cc", name=f"cc{ic}")
                nc.vector.tensor_tensor(out=cc[:], in0=c1[:], in1=c2[:],
                                        op=mybir.AluOpType.add)
                tcc = stpool.tile([128, 256], f32, tag="tcc", name=f"tcc{ic}")
                nc.scalar.activation(out=tcc[:], in_=cc[:], func=Tanh)
                nc.vector.tensor_tensor(out=w_ev[:, ic, :], in0=so[:],
                                        in1=tcc[:], op=mybir.AluOpType.mult)

            def emit_wpw():
                wpw = pp.tile([128, 256], f16)       # Wp @ W
                wp_ps = psg.tile([128, HID], f32, space="PSUM", tag="ops",
                                 name="wp_ps")
                nc.tensor.matmul(out=wp_ps[:], lhsT=wpt[:, 0, :], rhs=w_ev[:, 0, :],
                                 start=True, stop=False)
                nc.tensor.matmul(out=wp_ps[:], lhsT=wpt[:, 1, :], rhs=w_ev[:, 1, :],
                                 start=False, stop=True)
                nc.vector.tensor_copy(out=wpw[:], in_=wp_ps[:])
                bp_ps = psg.tile([128, HID], f32, space="PSUM", tag="ops",
                                 name="bp_ps")
                nc.tensor.matmul(out=bp_ps[0:1, :], lhsT=bp_c[:, 0, :],
                                 rhs=w_ev[:, 0, :], start=True, stop=False)
                nc.tensor.matmul(out=bp_ps[0:1, :], lhsT=bp_c[:, 1, :],
                                 rhs=w_ev[:, 1, :], start=False, stop=True)
                nc.vector.tensor_copy(out=bb[0:1, :], in_=bp_ps[0:1, :])
                return wpw

            def emit_final(g, accs, wpw):
                xagg = fpool.tile([128, 128], f16, tag="xagg", name=f"xagg{g}")
                ti = 0 if g < 4 else (1 if g < 8 else 2)
                acc, gg = accs[ti], g - (0, 4, 8)[ti]
                nc.vector.tensor_copy(
                    out=xagg[:], in_=acc[:, 128 * gg:128 * (gg + 1)])
                ops = psg.tile([128, HID], f32, space="PSUM", tag="ops",
                               name=f"ops{g}")
                ds = slice(128 * g, 128 * (g + 1))
                nc.tensor.matmul(out=ops[:], lhsT=sd[:, ds], rhs=bb[:],
                                 start=True, stop=False)
                nc.tensor.matmul(out=ops[:], lhsT=xagg[:], rhs=wpw[:],
                                 start=False, stop=True)
                orow = fpool.tile([128, HID], f16, tag="orow", name=f"orow{g}")
                nc.scalar.activation(out=orow[:], in_=ops[:],
                                     func=mybir.ActivationFunctionType.Copy,
                                     scale=dcol[:, g:g + 1])
                nc.sync.dma_start(
                    out=out_t.rearrange("(g p) h -> g p h", p=128)[g],
                    in_=orow[:],
                )

            # ---------- main: DoubleRow pass over 40 src rank pairs ---------
            acc_a = psacc_a.tile([128, 512], f32, space="PSUM", tag="acc_a")
            acc_b = psacc_b.tile([128, 512], f32, space="PSUM", tag="acc_b")
            acc_c = psacc_c.tile([128, 256], f32, space="PSUM", tag="acc_c")
            TILES = [(0, 512, acc_a), (512, 512, acc_b), (1024, 256, acc_c)]
            gp0 = gp1 = None
            wpw = None
            for pr in range(NPAIR):
                if pr == GATES_AFTER:
                    gp0 = emit_gates(0)
                if pr == GATES_AFTER + 8:
                    emit_lstm_post(gp0, 0)
                    gp1 = emit_gates(1)
                if pr == GATES_AFTER + 16:
                    emit_lstm_post(gp1, 1)
                if pr == GATES_AFTER + 20:
                    wpw = emit_wpw()
                r = 2 * pr
                k = bisect.bisect_right(CSTART, r) - 1
                jj = r - CSTART[k]
                sz = CHUNKS[k]
                sch = schunks[k]
                srow = sch[:, 0:sz * NPC].rearrange("p (j c) -> p j c", c=NPC)
                last = pr == NPAIR - 1
                last_hl = 0 if NPAIR - 1 in DROP_LO else 1
                for hl in range(2):
                    if hl == 1 and pr in DROP_LO:
                        continue
                    xb = sz * NPC + hl * sz * 128
                    x_t = sch[:, xb + jj * 128:xb + (jj + 2) * 128] \
                        .rearrange("p (j c) -> p j c", c=128)
                    for ti, (c0, w, acc) in enumerate(TILES):
                        nc.tensor.matmul(
                            out=acc[:],
                            lhsT=x_t,
                            rhs=srow[:, jj:jj + 2, c0:c0 + w],
                            start=(pr == 0 and hl == 0),
                            stop=(last and hl == last_hl),
                            perf_mode=DR,
                        )

            for g in range(NGRP):
                emit_final(g, (acc_a, acc_b, acc_c), wpw)

    nc.compile()
    return nc


def _preprocess(edge_index):
    """Host-side: degree norms, per-core fp8 count matrices (self loops in)."""
    src = np.asarray(edge_index[0], dtype=np.int64)
    dst = np.asarray(edge_index[1], dtype=np.int64)
    deg = np.bincount(dst, minlength=N_NODES).astype(np.float64) + 1.0
    dinv = (1.0 / np.sqrt(deg)).astype(np.float32)

    # s2[d] = sum over in-edges of dinv[src], self loop included
    s2 = (np.bincount(dst, weights=dinv[src].astype(np.float64),
                      minlength=N_NODES) + dinv.astype(np.float64)).astype(np.float32)
    return dinv, s2, src, dst


LAST_RESULT = None


def kernel(x, edge_index, Wp, bp, W_ih, W_hh, b_ih, b_hh, initial_weight, b_gcn):
    global LAST_RESULT
    import ml_dtypes
    from concourse.bass_utils import run_bass_kernel_spmd

    f8 = ml_dtypes.float8_e4m3
    x = np.asarray(x, np.float32)
    Wp = np.asarray(Wp, np.float32)
    bp = np.asarray(bp, np.float32)
    W_ih = np.asarray(W_ih, np.float32)
    W_hh = np.asarray(W_hh, np.float32)
    b_ih = np.asarray(b_ih, np.float32)
    b_hh = np.asarray(b_hh, np.float32)
    initial_weight = np.asarray(initial_weight, np.float32)
    b_gcn = np.asarray(b_gcn, np.float32)
    assert x.shape == (N_NODES, IN_DIM)

    dinv, s2, src, dst = _preprocess(edge_index)

    if "nc" not in _cache:
        _cache["nc"] = _build_module()
    nc = _cache["nc"]

    # host pre-scales x rows by dinv[src]; hi+lo fp8 token tables
    xp = np.zeros((NP, IN_DIM), np.float32)
    xp[:N_NODES] = x * dinv[:, None]
    xp_t = np.ascontiguousarray(
        xp.reshape(RANKS, 128, IN_DIM).transpose(1, 0, 2))  # [128, R, 128]
    xhi = xp_t.astype(f8)
    xlo = (xp_t - xhi.astype(np.float32)).astype(f8)

    def swi(xq):
        # DoubleRowSwInterleave weight layout per rank pair (A, B):
        # flat cols [A_127, B_127, A_126, B_126, ..., A_0, B_0]
        pq = xq.reshape(128, NPAIR, 2, 128)
        o = np.empty((128, NPAIR, 256), xq.dtype)
        o[:, :, 0::2] = pq[:, :, 0, ::-1]
        o[:, :, 1::2] = pq[:, :, 1, ::-1]
        return o

    xhi_swi = swi(xhi)
    xlo_swi = swi(xlo)

    core = dst // NPC
    dloc = dst - core * NPC

    wsumT = np.ascontiguousarray((W_ih + W_hh).T).astype(np.float16)
    bsum = (b_ih + b_hh).reshape(1, -1).astype(np.float16)

    s2p = np.zeros(NP, np.float32)
    s2p[:N_NODES] = s2
    drip = np.zeros(NP, np.float32)
    drip[:N_NODES] = 1.0 / dinv
    dlocp = np.zeros(NP, np.float32)
    dlocp[:N_NODES] = dinv

    shared = {
        "wsumT": wsumT,
        "bsum": bsum,
        "IW": initial_weight,
        "IWT": np.ascontiguousarray(initial_weight.T).astype(np.float16),
        "WpT": np.ascontiguousarray(Wp.T).astype(np.float16),
        "bp_col": np.ascontiguousarray(bp.reshape(-1, 1)).astype(np.float16),
        "b_gcn": b_gcn.reshape(1, -1).astype(np.float16),
        "ones_row": np.ones((1, 128), np.float16),
    }
    in_maps = []
    for c in range(M):
        m = core == c
        flat = src[m] * NPC + dloc[m]
        cnt = np.bincount(flat, minlength=NP * NPC)
        d0, d1 = c * NPC, min((c + 1) * NPC, N_NODES)
        dd = np.arange(d0, d1, dtype=np.int64)
        cnt[dd * NPC + (dd - d0)] += 1     # self loops
        sc = cnt.reshape(RANKS, 128, NPC).transpose(1, 0, 2)  # [128, R, NPC]
        sc8 = sc.astype(f8)
        stream = np.empty((128, RANKS * RW), f8)
        for k, sz in enumerate(CHUNKS):
            r0 = CSTART[k]
            base = r0 * RW
            ns = sz * NPC
            stream[:, base:base + ns] = sc8[:, r0:r0 + sz, :].reshape(128, ns)
            stream[:, base + ns:base + ns + sz * 128] = \
                xhi[:, r0:r0 + sz, :].reshape(128, sz * 128)
            stream[:, base + ns + sz * 128:base + sz * RW] = \
                xlo[:, r0:r0 + sz, :].reshape(128, sz * 128)
        sl = slice(c * NPC, (c + 1) * NPC)
        in_maps.append({
            **shared,
            "stream": stream,
            "sd_rows": np.stack([s2p[sl], drip[sl]]).astype(np.float16),
            "dinv_col": np.ascontiguousarray(
                dlocp[sl].reshape(NGRP, 128).T),
        })

    res = run_bass_kernel_spmd(nc, in_maps, list(range(M)))
    LAST_RESULT = res

    out = np.empty((N_NODES, HID), np.float32)
    for c in range(M):
        d0, d1 = c * NPC, min((c + 1) * NPC, N_NODES)
        out[d0:d1] = res.results[c]["out"][:d1 - d0].astype(np.float32)
    return out
